# revision 28
# baseline (speedup 1.0000x reference)
"""Trainium2 Bass kernel for nn_ExpansionContrastModule.

Strategy: pure data parallel over 8 cores; each core processes half of one
batch image (128 of 256 rows), with a 3-row halo so the dilated contrast
convs and the 3x3 mas conv need no cross-core traffic.

v3: cen is loaded ONCE as bf16 (8 row-chunks, SBUF-resident all kernel),
feeding both the input 1x1-conv matmuls and the final gate multiply; the
output is written bf16 and widened on the host.  Per macro-half, x / x_odd
/ mas are built once; the contrast stage and everything downstream run per
QUARTER (16 image rows, free-size 1024) giving 8 independent dependency
chains that keep DVE/Pool/ACT fed.  The scale2-weighted min/mean/max
combine across the two shifts is folded into three accumulating bc-conv
matmuls (y = (v0+v1/2)*(t1+t3) + (v2-v0)*max(t1,t3)), so only max(t1,t3)
is materialized.  All ACT ops stick to the {identity, sigmoid, copy}
activation table (SiLU = y*sigmoid(y)) so only one table load is issued.
Elementwise work is split across DVE/Pool via the ASSIGN table, tuned
against TimelineSim.
"""
import sys
import ml_dtypes
import numpy as np

sys.path.insert(0, "/opt/trn_rl_repo")

import concourse.bass as bass
import concourse.bacc as bacc
import concourse.mybir as mybir
from concourse.tile import TileContext
from concourse.bass_utils import run_bass_kernel_spmd

F32 = mybir.dt.float32
BF = mybir.dt.bfloat16
AF = mybir.ActivationFunctionType
ALU = mybir.AluOpType

N_CORES = 8
C = 128        # input channels
CR = 16        # reduced channels
H = W = 256
CH = 128       # rows per core (half an image)
MH = 2         # macro-halves per core
HB = 64        # rows per macro-half
G = 8          # row-groups per macro-half
GR = 8         # rows per group
XR = GR + 6    # x tile rows (3-row halo each side)
XP = 4         # x tile left/right col pad (4 for bf16 4B alignment)
XW = W + 2 * XP  # x tile cols
NQ = 2         # quarters per macro-half
QR = GR // NQ  # group-rows per quarter (4)
QF = QR * W    # free elems per quarter (1024)
HF = GR * W    # free elems per half (2048)

BN_EPS = 1e-5

# cbf column layout
CB_WIN = 0          # 0:1024    w_in_blk
CB_BCA = 1024       # 1024:1152 (v0+v1/2) * bc_blk
CB_BCD = 1152       # 1152:1280 (v2-v0)   * bc_blk
CB_WOUT = 1280      # 1280:1288 wout_lhsT
CB_MAS = 1288       # 1288:1296 mas_lhsT (rows 0:72)
CB_BCAST = 1296     # 1296:2320 bcast_lhsT (rows 0:9)
CB_W = 2320

# cf32 scalar column indices (cols 5..)
S_W0, S_W1_4, S_W2 = 0, 1, 2
S_G0, S_G1, S_G2 = 3, 4, 5
S_BOUT, S_MB1, S_MW2, S_MB2 = 6, 7, 8, 9
CF_W = 16

_CACHE = {}

# Engine assignment per op site, cycled by call index ('dve' -> nc.vector,
# 'pool' -> nc.gpsimd).
# GPSIMD (pool) only lowers add/subtract/mult tensor_tensor and
# tensor_scalar — min/max and scalar_tensor_tensor are DVE-only.
ASSIGN = {
    'sub':      ('dve',),                    # 8 per shift-quarter
    'mult':     ('dve',),             # 4 per shift-quarter
    'pair_mm':  ('dve',),                    # m01,M01,m23,M23 (DVE only)
    'root_mm':  ('dve',),                    # min4,max4 (DVE only)
    'tree_add': ('pool',),                   # s01,s23,sum4
    'comb_ts':  ('act',),              # q1,qa,qb scalar muls
    'mx':       ('dve',),
    'zmul':     ('dve',),                   # silu product for bc output
    'gate':     ('dve',),                    # ct
    'ge':       ('pool',),                   # s1*mm
    'g9add':    ('pool',),                   # gate9 = ct + ge
    'final':    ('stage-dve', 'stage-dve', 'stage-pool', 'stage-dve',
                 'stage-dve', 'stage-dve', 'stage-pool', 'stage-dve'),
}


def _fl(t):
    """Flatten the two free dims of a [P, a, b] tile AP into [P, a*b]."""
    return t[:, :, :].rearrange("p a b -> p (a b)")


def build_nc(loop_reps=0):
    nc = bacc.Bacc("TRN2", target_bir_lowering=False, debug=False,
                   num_devices=N_CORES)
    cen_b = nc.dram_tensor("cen_bf", [C, CH + 6, W], BF, kind="ExternalInput")
    mas_p = nc.dram_tensor("mas", [CH + 8, W], BF, kind="ExternalInput")
    cbf_d = nc.dram_tensor("cbf", [C, CB_W], BF, kind="ExternalInput")
    cf32_d = nc.dram_tensor("cf32", [C, CF_W], F32, kind="ExternalInput")
    out_p = nc.dram_tensor("out", [C, CH, W], BF, kind="ExternalOutput")

    def ENG(site, i=0):
        kind = ASSIGN[site][i % len(ASSIGN[site])]
        return nc.vector if kind == 'dve' else nc.gpsimd

    with TileContext(nc) as tc:
        import contextlib
        _stk = contextlib.ExitStack()
        with _stk:
            cpool = _stk.enter_context(tc.tile_pool(name="const", bufs=1))
            cenpool = _stk.enter_context(tc.tile_pool(name="cen", bufs=1))
            xpool = _stk.enter_context(tc.tile_pool(name="x", bufs=2))
            xopool = _stk.enter_context(tc.tile_pool(name="xo", bufs=1))
            dpool = _stk.enter_context(tc.tile_pool(name="d", bufs=6))
            opool = _stk.enter_context(tc.tile_pool(name="o", bufs=4))
            trpool = _stk.enter_context(tc.tile_pool(name="tr", bufs=8))
            tspool = _stk.enter_context(tc.tile_pool(name="ts", bufs=4))
            typool = _stk.enter_context(tc.tile_pool(name="ty", bufs=2))
            zpool = _stk.enter_context(tc.tile_pool(name="z", bufs=2))
            qpool = _stk.enter_context(tc.tile_pool(name="q8", bufs=4))
            mqpool = _stk.enter_context(tc.tile_pool(name="mq", bufs=2))
            mspool = _stk.enter_context(tc.tile_pool(name="msil", bufs=2))
            g9pool = _stk.enter_context(tc.tile_pool(name="g9", bufs=2))
            mas9pool = _stk.enter_context(tc.tile_pool(name="m9", bufs=1))
            gatepool = _stk.enter_context(tc.tile_pool(name="gb", bufs=3))
            outqpool = _stk.enter_context(tc.tile_pool(name="oq", bufs=2))
            pspool = _stk.enter_context(tc.tile_pool(name="ps", bufs=2, space="PSUM"))
            pbpool = _stk.enter_context(tc.tile_pool(name="pb", bufs=2, space="PSUM"))
            pupool = _stk.enter_context(tc.tile_pool(name="pu", bufs=2, space="PSUM"))
            pgpool = _stk.enter_context(tc.tile_pool(name="pg", bufs=2, space="PSUM"))

            # ---- constants + resident cen (8 row-chunks) ----
            cbf_sb = cpool.tile([C, CB_W], BF, tag="c_bf")
            cf32_sb = cpool.tile([C, CF_W], F32, tag="c_f32")
            nc.sync.dma_start(out=cbf_sb[:], in_=cbf_d[:])
            nc.sync.dma_start(out=cf32_sb[:], in_=cf32_d[:])

            # cen loads are group-strided: x-conv row r accumulates over all
            # 8 row-groups (rows base+3+8g+r), so chunk r carries exactly
            # those 8 rows and row r's matmuls start after one ~0.5MB DMA.
            cen_sb = cenpool.tile([C, CH + 6, W], BF, tag="cen")
            nc.sync.dma_start(out=cen_sb[:, 0:3, :], in_=cen_b[:, 0:3, :])

            def cen_chunks(mh):
                base = mh * HB
                sv = cen_sb[:, base + 3:base + 67, :].rearrange(
                    "c (g x) w -> c g x w", x=GR)
                bv = cen_b[:, base + 3:base + 67, :].rearrange(
                    "c (g x) w -> c g x w", x=GR)
                for r in range(GR):
                    nc.sync.dma_start(out=sv[:, :, r, :], in_=bv[:, :, r, :])

            cen_chunks(0)
            # mh0's bottom-edge rows, early (mh1's chunks rewrite them later)
            nc.sync.dma_start(out=cen_sb[:, 67:70, :], in_=cen_b[:, 67:70, :])
            cen_chunks(1)
            nc.sync.dma_start(out=cen_sb[:, CH + 3:CH + 6, :],
                              in_=cen_b[:, CH + 3:CH + 6, :])

            def sc(col, p=C):
                return cf32_sb[0:p, 5 + col:6 + col]

            def build_x(mh):
                """x[16g+c, 3+r, XP+w] = w_in . cen(base+8g+r) + b_in."""
                base = mh * HB
                x = xpool.tile([C, XR, XW], BF, tag="x")
                nc.gpsimd.memset(x[:, :, 0:XP], 0.0)
                nc.gpsimd.memset(x[:, :, W + XP:XW], 0.0)

                for r in range(GR):
                    pxc = pspool.tile([C, W], F32, tag="ps")
                    for g in range(G):
                        nc.tensor.matmul(
                            pxc[:], cbf_sb[:, g * C:(g + 1) * C],
                            cen_sb[:, base + 3 + g * GR + r, :],
                            start=(g == 0), stop=(g == G - 1))
                    nc.scalar.activation(
                        x[:, 3 + r, XP: XP + W], pxc[:], AF.Identity,
                        bias=cf32_sb[:, 0:1], scale=1.0)

                for j in range(3):
                    pe = pspool.tile([C, W], F32, tag="ps")
                    nc.tensor.matmul(pe[:], cbf_sb[:, 0:C],
                                     cen_sb[:, base + j, :], start=True,
                                     stop=True)
                    bt_c = 1 if mh == 0 else 0
                    nc.scalar.activation(
                        x[0:CR, j, XP: XP + W], pe[0:CR, :], AF.Identity,
                        bias=cf32_sb[0:CR, bt_c:bt_c + 1], scale=1.0)
                    pe2 = pspool.tile([C, W], F32, tag="ps")
                    nc.tensor.matmul(pe2[:], cbf_sb[:, (G - 1) * C:G * C],
                                     cen_sb[:, base + 67 + j, :], start=True,
                                     stop=True)
                    # start partition must be a multiple of 32; rows 96:112
                    # get junk here and are re-written by the interior-halo
                    # DMA below (WAW-ordered by Tile).
                    bb_c = 2 if mh == MH - 1 else 0
                    nc.scalar.activation(
                        x[96: C, 11 + j, XP: XP + W], pe2[96: C, :],
                        AF.Identity, bias=cf32_sb[96:C, bb_c:bb_c + 1],
                        scale=1.0)

                # interior halos between groups via partition-shifted SBUF DMA
                nc.sync.dma_start(out=x[CR:C, 0:3, XP:XP + W],
                                  in_=x[0:C - CR, GR:GR + 3, XP:XP + W])
                nc.sync.dma_start(out=x[0:C - CR, GR + 3:GR + 6, XP:XP + W],
                                  in_=x[CR:C, 3:6, XP:XP + W])

                # x_odd[c] = x[c+1]: keeps odd-column reads 4B-aligned.
                # Split at row 11 so quarter 0 (x_odd rows 0..10) doesn't
                # wait on the late edge rows 11..13.
                x_odd = xopool.tile([C, XR, XW], BF, tag="xo")
                nc.vector.tensor_copy(_fl(x_odd)[:, 0:11 * XW - 1],
                                      _fl(x)[:, 1:11 * XW])
                nc.vector.tensor_copy(_fl(x_odd)[:, 11 * XW:XR * XW - 1],
                                      _fl(x)[:, 11 * XW + 1:XR * XW])
                return x, x_odd

            def build_mas(mh):
                """mm over the half: sigmoid(mw2*silu(conv3x3(mas)+mb1)+mb2)."""
                base = mh * HB
                mas9 = mas9pool.tile([72, GR, W], BF, tag="m9")
                nc.gpsimd.memset(mas9[:, :, 0:1], 0.0)
                nc.gpsimd.memset(mas9[:, :, W - 1:W], 0.0)
                t = 0
                for dy in (-1, 0, 1):
                    for dx in (-1, 0, 1):
                        cs, cd = max(0, dx), max(0, -dx)
                        n = W - abs(dx)
                        msrc = mas_p[base + dy + 1: base + dy + 1 + 64, :]
                        msrc = msrc.rearrange("(g x) w -> g x w", x=GR)
                        nc.sync.dma_start(
                            out=mas9[G * t:G * (t + 1), :, cd:cd + n],
                            in_=msrc[:, :, cs:cs + n])
                        t += 1
                mm_q = mqpool.tile([G, HF], BF, tag="mq")
                m_t = mspool.tile([G, HF], BF, tag="qs")
                m_s = mspool.tile([G, HF], BF, tag="qs")
                m9f = _fl(mas9)
                for c2 in range(4):
                    cs2 = slice(512 * c2, 512 * (c2 + 1))
                    pm = pupool.tile([G, 512], F32, tag="pu")
                    nc.tensor.matmul(pm[:], cbf_sb[0:72, CB_MAS:CB_MAS + 8],
                                     m9f[:, cs2], start=True, stop=True)
                    nc.scalar.activation(m_t[:, cs2], pm[:], AF.Identity,
                                         bias=sc(S_MB1, G), scale=1.0)
                    nc.scalar.activation(m_s[:, cs2], pm[:], AF.Sigmoid,
                                         bias=sc(S_MB1, G), scale=1.0)
                nc.vector.tensor_tensor(m_t[:], m_t[:], m_s[:], ALU.mult)
                nc.scalar.activation(mm_q[:], m_t[:], AF.Sigmoid,
                                     bias=sc(S_MB2, G), scale=sc(S_MW2, G))
                return mm_q

            def emit_quarter(mh, qq, x, x_odd, mm_q):
                base = mh * HB
                j0 = 3 + QR * qq
                last_q = (mh == MH - 1 and qq == NQ - 1)

                def xin(dh, dw):
                    if dw % 2 == 0:
                        return x[:, j0 + dh:j0 + QR + dh,
                                 XP + dw:XP + W + dw]
                    return x_odd[:, j0 + dh:j0 + QR + dh,
                                 XP + dw - 1:XP + W + dw - 1]

                x_c = x[:, j0:j0 + QR, XP:XP + W]

                # ---- contrast stage ----
                ts_tiles = []
                for si, s in enumerate((1, 3)):
                    dirs = [(-s, -s), (-s, 0), (-s, s), (0, -s)]

                    def make_o(i):
                        dh, dw = dirs[i]
                        d1 = dpool.tile([C, QR, W], BF, tag="d")
                        ENG('sub', 2 * i).tensor_tensor(
                            d1[:], x_c, xin(dh, dw), ALU.subtract)
                        d2 = dpool.tile([C, QR, W], BF, tag="d")
                        ENG('sub', 2 * i + 1).tensor_tensor(
                            d2[:], x_c, xin(-dh, -dw), ALU.subtract)
                        o = opool.tile([C, QF], BF, tag="o")
                        ENG('mult', 2 * si + i).tensor_tensor(
                            o[:], _fl(d1), _fl(d2), ALU.mult)
                        return o

                    o0, o1 = make_o(0), make_o(1)
                    m01 = trpool.tile([C, QF], BF, tag="tr")
                    ENG('pair_mm', 0).tensor_tensor(m01[:], o0[:], o1[:],
                                                    ALU.min)
                    M01 = trpool.tile([C, QF], BF, tag="tr")
                    ENG('pair_mm', 1).tensor_tensor(M01[:], o0[:], o1[:],
                                                    ALU.max)
                    s01 = trpool.tile([C, QF], BF, tag="tr")
                    (nc.vector if last_q else ENG('tree_add', 0)).tensor_tensor(s01[:], o0[:], o1[:],
                                                     ALU.add)
                    o2, o3 = make_o(2), make_o(3)
                    m23 = trpool.tile([C, QF], BF, tag="tr")
                    ENG('pair_mm', 2).tensor_tensor(m23[:], o2[:], o3[:],
                                                    ALU.min)
                    min4 = trpool.tile([C, QF], BF, tag="tr")
                    ENG('root_mm', 0).tensor_tensor(min4[:], m01[:], m23[:],
                                                    ALU.min)
                    M23 = trpool.tile([C, QF], BF, tag="tr")
                    ENG('pair_mm', 3).tensor_tensor(M23[:], o2[:], o3[:],
                                                    ALU.max)
                    max4 = trpool.tile([C, QF], BF, tag="tr")
                    ENG('root_mm', 1).tensor_tensor(max4[:], M01[:], M23[:],
                                                    ALU.max)
                    s23 = trpool.tile([C, QF], BF, tag="tr")
                    (nc.vector if last_q else ENG('tree_add', 1)).tensor_tensor(s23[:], o2[:], o3[:],
                                                     ALU.add)
                    sum4 = trpool.tile([C, QF], BF, tag="tr")
                    (nc.vector if last_q else ENG('tree_add', 2)).tensor_tensor(sum4[:], s01[:], s23[:],
                                                     ALU.add)

                    # t_s = w0*min4 + (w1/4)*sum4 + w2*max4
                    def ts_mul(dst, src, col, i):
                        kind = ASSIGN['comb_ts'][i % len(ASSIGN['comb_ts'])]
                        if last_q:
                            kind = 'dve'
                        if kind == 'act':
                            nc.scalar.activation(dst, src, AF.Identity,
                                                 bias=0.0, scale=sc(col))
                        elif kind == 'pool':
                            nc.gpsimd.tensor_scalar_mul(dst, src, sc(col))
                        else:
                            nc.vector.tensor_scalar_mul(dst, src, sc(col))

                    q1 = trpool.tile([C, QF], BF, tag="tr")
                    ts_mul(q1[:], sum4[:], S_W1_4, 3 * si)
                    qa = trpool.tile([C, QF], BF, tag="tr")
                    ts_mul(qa[:], min4[:], S_W0, 3 * si + 1)
                    tq = trpool.tile([C, QF], BF, tag="tr")
                    nc.vector.tensor_tensor(tq[:], qa[:], q1[:], ALU.add)
                    qb = trpool.tile([C, QF], BF, tag="tr")
                    ts_mul(qb[:], max4[:], S_W2, 3 * si + 2)
                    t_s = tspool.tile([C, QF], BF, tag="ts")
                    nc.vector.tensor_tensor(t_s[:], qb[:], tq[:], ALU.add)
                    ts_tiles.append(t_s)

                t1, t3 = ts_tiles
                mx = typool.tile([C, QF], BF, tag="tymx")
                ENG('mx').tensor_tensor(mx[:], t1[:], t3[:], ALU.max)

                # ---- bc 1x1 conv + BN + SiLU ----
                z_q = zpool.tile([C, QF], BF, tag="z")
                for c2 in range(2):
                    lo = 512 * c2
                    pbc = pbpool.tile([C, 512], F32, tag="pb")
                    nc.tensor.matmul(pbc[:], cbf_sb[:, CB_BCA:CB_BCA + C],
                                     t1[:, lo:lo + 512], start=True, stop=False)
                    nc.tensor.matmul(pbc[:], cbf_sb[:, CB_BCA:CB_BCA + C],
                                     t3[:, lo:lo + 512], start=False, stop=False)
                    nc.tensor.matmul(pbc[:], cbf_sb[:, CB_BCD:CB_BCD + C],
                                     mx[:, lo:lo + 512], start=False, stop=True)
                    z_t = zpool.tile([C, 512], BF, tag="zt")
                    nc.scalar.activation(z_t[:], pbc[:], AF.Identity,
                                         bias=cf32_sb[:, 4:5],
                                         scale=cf32_sb[:, 3:4])
                    z_s = zpool.tile([C, 512], BF, tag="zt")
                    nc.scalar.activation(z_s[:], pbc[:], AF.Sigmoid,
                                         bias=cf32_sb[:, 4:5],
                                         scale=cf32_sb[:, 3:4])
                    (nc.vector if last_q else ENG('zmul', c2)).tensor_tensor(z_q[:, lo:lo + 512],
                                                  z_t[:], z_s[:], ALU.mult)

                # ---- w_out 1x1 + sigmoid -> om ----
                om_q = qpool.tile([G, QF], BF, tag="q8")
                for c2 in range(2):
                    pu = pupool.tile([G, 512], F32, tag="pu")
                    nc.tensor.matmul(pu[:], cbf_sb[:, CB_WOUT:CB_WOUT + 8],
                                     z_q[:, 512 * c2:512 * (c2 + 1)],
                                     start=True, stop=True)
                    nc.scalar.activation(om_q[:, 512 * c2:512 * (c2 + 1)],
                                         pu[:], AF.Sigmoid,
                                         bias=sc(S_BOUT, G), scale=1.0)

                # ---- gate rows: g9 = s1*mm + om*(s2*mm+s0)
                # (s3 enters via all-ones row 8 and the bcast weights) ----
                mm_sl = mm_q[:, QF * qq:QF * (qq + 1)]
                bt = qpool.tile([G, QF], BF, tag="q8")
                nc.vector.tensor_scalar(bt[:], mm_sl, sc(S_G2, G),
                                        sc(S_G0, G), ALU.mult, ALU.add)
                ct = qpool.tile([G, QF], BF, tag="q8")
                ENG('gate', 0).tensor_tensor(ct[:], om_q[:], bt[:], ALU.mult)
                ge = qpool.tile([G, QF], BF, tag="q8")
                ge_eng = nc.vector if last_q else ENG('ge')
                ge_eng.tensor_scalar_mul(ge[:], mm_sl, sc(S_G1, G))
                gate9 = g9pool.tile([9, QF], BF, tag="g9")
                nc.gpsimd.memset(gate9[:], 1.0)
                g9_eng = nc.vector if last_q else ENG('g9add')
                g9_eng.tensor_tensor(gate9[0:G, :], ct[:], ge[:], ALU.add)

                # ---- broadcast gate (PE) + final multiply + store ----
                outq = None
                for g in range(G):
                    if g % 4 == 0:
                        outq = outqpool.tile([C, 4, QR, W], BF, tag="oq")
                    mode = ASSIGN['final'][g]
                    if last_q:
                        # tail: skip ACT staging, multiply straight from PSUM
                        mode = 'direct'
                    r0 = base + GR * g + QR * qq
                    cen_flat = cen_sb[:, r0 + 3:r0 + 3 + QR, :].rearrange(
                        "p a b -> p (a b)")
                    oq_flat = outq[:, g % 4, :, :].rearrange(
                        "p a b -> p (a b)")
                    if mode == 'direct':
                        for c2 in range(2):
                            lo = 512 * c2
                            pg = pgpool.tile([C, 512], F32, tag="pg")
                            nc.tensor.matmul(
                                pg[:],
                                cbf_sb[0:9,
                                       CB_BCAST + g * C:CB_BCAST + (g + 1) * C],
                                gate9[:, lo:lo + 512], start=True, stop=True)
                            nc.vector.tensor_tensor(
                                oq_flat[:, lo:lo + 512],
                                cen_flat[:, lo:lo + 512], pg[:], ALU.mult)
                    else:
                        gate_sb = gatepool.tile([C, QF], BF, tag="gb")
                        for c2 in range(2):
                            lo = 512 * c2
                            pg = pgpool.tile([C, 512], F32, tag="pg")
                            nc.tensor.matmul(
                                pg[:],
                                cbf_sb[0:9,
                                       CB_BCAST + g * C:CB_BCAST + (g + 1) * C],
                                gate9[:, lo:lo + 512], start=True, stop=True)
                            nc.scalar.activation(gate_sb[:, lo:lo + 512],
                                                 pg[:], AF.Identity,
                                                 scale=1.0)
                        eng = nc.vector if mode == 'stage-dve' else nc.gpsimd
                        eng.tensor_tensor(oq_flat[:], cen_flat, gate_sb[:],
                                          ALU.mult)
                    if g % 4 == 3:
                        # one DMA per 4 groups: rows 4qq..4qq+4 of each
                        half_v = out_p[:, base:base + HB, :].rearrange(
                            "c (g x) w -> c g x w", x=GR)
                        nc.sync.dma_start(
                            out=half_v[:, g - 3:g + 1,
                                       QR * qq:QR * (qq + 1), :],
                            in_=outq[:])

            import contextlib as _ctx
            rep_ctx = (tc.For_i(0, loop_reps, 1) if loop_reps
                       else _ctx.nullcontext())
            with rep_ctx:
                for mh in range(MH):
                    x, x_odd = build_x(mh)
                    mm_q = build_mas(mh)
                    for qq in range(NQ):
                        emit_quarter(mh, qq, x, x_odd, mm_q)
    nc.compile()
    return nc


def _softmax(v):
    e = np.exp(v - v.max())
    return e / e.sum()


def _prep_consts(inp):
    w = _softmax(inp['scale1'])
    v = _softmax(inp['scale2'])
    s3 = _softmax(inp['scale3'])
    inv = inp['bn_gamma'] / np.sqrt(inp['bn_var'] + BN_EPS)
    bnb = inp['bn_beta'] - inp['bn_mean'] * inv

    w_in_blk = np.zeros((C, G, C), np.float32)
    for g in range(G):
        w_in_blk[:, g, CR * g:CR * (g + 1)] = inp['w_in'].T
    b_in_t = np.tile(inp['b_in'], G)[:, None].astype(np.float32)
    bc_blk = np.kron(np.eye(G), inp['bc_w'].T)
    a_w = v[0] + v[1] / 2.0
    d_w = v[2] - v[0]
    wout_lhsT = np.kron(np.eye(G), inp['w_out'][0][:, None]).astype(np.float32)
    k_flat = inp['mas_w1'][0, 0].reshape(9)
    mas_lhsT = np.kron(k_flat[:, None], np.eye(G)).astype(np.float32)
    bcast = np.zeros((9, G, C), np.float32)
    for g in range(G):
        bcast[g, g, :] = 1.0
    bcast[8, :, :] = s3[3]

    scal = np.zeros((C, 10), np.float32)
    vals = [w[0], w[1] / 4.0, w[2], s3[0], s3[1], s3[2],
            inp['b_out'][0], inp['mas_b1'][0],
            inp['mas_w2'][0, 0], inp['mas_b2'][0]]
    scal[:] = np.asarray(vals, np.float32)[None, :]

    cbf = np.zeros((C, CB_W), np.float32)
    cbf[:, CB_WIN:CB_WIN + 1024] = w_in_blk.reshape(C, G * C)
    cbf[:, CB_BCA:CB_BCA + C] = (a_w * bc_blk).astype(np.float32)
    cbf[:, CB_BCD:CB_BCD + C] = (d_w * bc_blk).astype(np.float32)
    cbf[:, CB_WOUT:CB_WOUT + 8] = wout_lhsT
    cbf[0:72, CB_MAS:CB_MAS + 8] = mas_lhsT
    cbf[0:9, CB_BCAST:CB_BCAST + 1024] = bcast.reshape(9, G * C)

    cf32 = np.zeros((C, CF_W), np.float32)
    cf32[:, 0:1] = b_in_t
    cf32[:, 3:4] = np.tile(inv, G)[:, None].astype(np.float32)
    cf32[:, 4:5] = np.tile(bnb, G)[:, None].astype(np.float32)
    cf32[:, 5:15] = scal
    return {'cbf': cbf.astype(ml_dtypes.bfloat16), 'cf32': cf32,
            'b_in_t': b_in_t}


def _make_in_maps(inp, consts):
    in_maps = []
    for core in range(N_CORES):
        b, hf = core // 2, core % 2
        r0 = CH * hf
        cen_pad = np.pad(inp['cen'][b], ((0, 0), (3, 3), (0, 0)))
        mas_pad = np.pad(inp['mas'][b, 0], ((1, 9), (0, 0)))
        cen_core = np.ascontiguousarray(cen_pad[:, r0:r0 + CH + 6, :])
        cf32 = consts['cf32'].copy()
        if hf != 0:
            cf32[:, 1:2] = consts['b_in_t']
        if hf != 1:
            cf32[:, 2:3] = consts['b_in_t']
        m = {
            'cen_bf': cen_core.astype(ml_dtypes.bfloat16),
            'mas': np.ascontiguousarray(
                mas_pad[r0:r0 + CH + 8, :]).astype(ml_dtypes.bfloat16),
            'cbf': consts['cbf'],
            'cf32': cf32,
        }
        in_maps.append(m)
    return in_maps


def run(inputs, trace=False):
    inp = {k: np.asarray(v) for k, v in inputs.items()}
    consts = _prep_consts(inp)

    if 'nc' not in _CACHE:
        _CACHE['nc'] = build_nc()
    nc = _CACHE['nc']

    in_maps = _make_in_maps(inp, consts)
    res = run_bass_kernel_spmd(nc, in_maps, list(range(N_CORES)), trace=trace)

    out = np.empty((4, C, H, W), np.float32)
    for core in range(N_CORES):
        b, hf = core // 2, core % 2
        out[b, :, CH * hf:CH * (hf + 1), :] = np.asarray(
            res.results[core]['out']).astype(np.float32)
    return out, res


def kernel(**inputs):
    return run(inputs)[0]


def bench(inputs, iters=30, reps=0):
    """Time repeated executions with device-resident inputs (no donation)."""
    import time
    import jax
    from jax.sharding import Mesh, PartitionSpec
    from jax.experimental.shard_map import shard_map
    from concourse import bass2jax

    inp = {k: np.asarray(v) for k, v in inputs.items()}
    consts = _prep_consts(inp)
    key = ('nc', reps)
    if key not in _CACHE:
        _CACHE[key] = build_nc(loop_reps=reps)
    nc = _CACHE[key]

    in_maps = _make_in_maps(inp, consts)

    bass2jax.install_neuronx_cc_hook()
    in_names, out_names, out_avals, zero_outs = [], [], [], []
    for alloc in nc.m.functions[0].allocations:
        if not isinstance(alloc, mybir.MemoryLocationSet):
            continue
        name = alloc.memorylocations[0].name
        pname = (nc.partition_id_tensor.name if nc.partition_id_tensor
                 else None)
        if alloc.kind == "ExternalInput":
            if name != pname:
                in_names.append(name)
        elif alloc.kind == "ExternalOutput":
            out_names.append(name)
            out_avals.append(jax.core.ShapedArray(
                tuple(alloc.tensor_shape), mybir.dt.np(alloc.dtype)))
            zero_outs.append(np.zeros(tuple(alloc.tensor_shape),
                                      mybir.dt.np(alloc.dtype)))
    n_params = len(in_names)
    all_names = in_names + out_names
    if nc.partition_id_tensor:
        all_names = all_names + [nc.partition_id_tensor.name]

    def _body(*args):
        operands = list(args)
        if nc.partition_id_tensor:
            operands.append(bass2jax.partition_id_tensor())
        outs = bass2jax._bass_exec_p.bind(
            *operands,
            out_avals=tuple(out_avals),
            in_names=tuple(all_names),
            out_names=tuple(out_names),
            lowering_input_output_aliases=(),
            sim_require_finite=True,
            sim_require_nnan=True,
            nc=nc,
        )
        return tuple(outs)

    devices = jax.devices()[:N_CORES]
    mesh = Mesh(np.asarray(devices), ("core",))
    nin = n_params + len(out_names)
    sharded = jax.jit(
        shard_map(_body, mesh=mesh,
                  in_specs=(PartitionSpec("core"),) * nin,
                  out_specs=(PartitionSpec("core"),) * len(out_names),
                  check_rep=False),
        donate_argnums=tuple(range(n_params, n_params + len(out_names))),
        keep_unused=True,
    )
    concat_in = [np.concatenate([in_maps[c][nm] for c in range(N_CORES)], 0)
                 for nm in in_names]
    concat_zero = [np.zeros((N_CORES * z.shape[0], *z.shape[1:]), z.dtype)
                   for z in zero_outs]
    sh = jax.sharding.NamedSharding(mesh, PartitionSpec("core"))
    dev_in = [jax.device_put(a, sh) for a in concat_in]
    prev = jax.device_put(concat_zero[0], sh)

    outs = sharded(*dev_in, prev)
    jax.block_until_ready(outs)
    result = np.asarray(outs[0]).copy()
    prev = outs[0]
    times = []
    for _ in range(iters):
        t0 = time.perf_counter()
        outs = sharded(*dev_in, prev)
        jax.block_until_ready(outs)
        times.append(time.perf_counter() - t0)
        prev = outs[0]

    full = np.empty((4, C, H, W), np.float32)
    arr = result.reshape(N_CORES, C, CH, W)
    for core in range(N_CORES):
        b, hf = core // 2, core % 2
        full[b, :, CH * hf:CH * (hf + 1), :] = arr[core].astype(np.float32)
    return full, times
